# revision 6
# baseline (speedup 1.0000x reference)
"""Trainium2 Bass kernel for the CoLES problem (GRU encoder + NT-Xent loss).

Strategy (8 NeuronCores, data-parallel over the K*B=1024 subsequences):
  * host: extract subsequence token ids (pure indexing); shard 128 rows/core.
  * phase 1 (per core): the one-hot input GEMM is an embedding gather from
    (W_ih.T + baked biases) via dma_gather (bf16 table in HBM, prefetched
    6 steps deep); 64 GRU steps with bf16 matmuls accumulated in fp32 PSUM.
    gx_r / gx_z are injected into PSUM with identity matmuls so the r/z
    gates are a single ACT sigmoid straight from PSUM; the gh matmuls are
    emitted gate-major so ps_r completes first; the n-gate/update/transpose
    tail is chunked 256+256 along H so next-step matmuls overlap the tail.
    Mean-pool+projection is folded into PSUM-accumulated matmuls of bf16
    pair/quad partial sums against proj_W.T/64 (once per 4 steps).
  * host glue: add proj_b, L2-normalize embeddings, build phase-2 operands.
  * phase 2 (per core): a 128x1024 block of the similarity matrix (fp32
    matmul), es=exp(sim/tau) with fused row-sum, fused row-sum of es*pos.
  * host: loss = mean(-log(num/(den+1e-10)+1e-10)) with has_pos handling.
"""
import os
import sys

sys.path.insert(0, "/opt/trn_rl_repo")

import numpy as np
import ml_dtypes

import concourse.bass as bass
import concourse.tile as tile
from concourse import bacc, mybir
from concourse.masks import make_identity
from concourse.bass_utils import run_bass_kernel_spmd

BF = mybir.dt.bfloat16
F32 = mybir.dt.float32
I16 = mybir.dt.int16

B, S, V, H, E, L, K = 512, 512, 1024, 512, 256, 64, 2
TAU = 0.1
NCORES = 8
N = K * B
NLOC = N // NCORES  # 128

SIG = mybir.ActivationFunctionType.Sigmoid
TANH = mybir.ActivationFunctionType.Tanh
EXP = mybir.ActivationFunctionType.Exp


def _build_phase1(steps=L, gx_bufs=8, tail_plan=(256, 256), ht_copy_dve=4,
                  demote_pool=100000):
    nc = bacc.Bacc("TRN2", target_bir_lowering=False, debug=False)
    table = nc.dram_tensor("table", [V, 3 * H], BF, kind="ExternalInput").ap()
    whht = nc.dram_tensor("whht", [H, 3 * H], BF, kind="ExternalInput").ap()
    bhhn = nc.dram_tensor("bhhn", [1, H], BF, kind="ExternalInput").ap()
    projwt = nc.dram_tensor("projwt", [H, E], BF, kind="ExternalInput").ap()
    idx = nc.dram_tensor("idx", [128, steps * 8], I16, kind="ExternalInput").ap()
    zraw = nc.dram_tensor("zraw", [NLOC, E], F32, kind="ExternalOutput").ap()

    chunks = []
    off = 0
    for t in tail_plan:
        chunks.append((off, t, list(range(off // 128, (off + t) // 128))))
        off += t

    with tile.TileContext(nc) as tc:
        with (
            tc.tile_pool(name="singles", bufs=1) as singles,
            tc.tile_pool(name="gx", bufs=gx_bufs) as gxp,
            tc.tile_pool(name="state", bufs=2) as statep,
            tc.tile_pool(name="gates", bufs=2) as gatep,
            tc.tile_pool(name="psg", bufs=2, space="PSUM") as psg,
            tc.tile_pool(name="psT", bufs=1, space="PSUM") as psT,
            tc.tile_pool(name="psE", bufs=1, space="PSUM") as psE,
        ):
            idx_sb = singles.tile([128, steps * 8], I16, tag="idx")
            nc.sync.dma_start(idx_sb[:], idx[:])
            whht_sb = []
            for c in range(4):
                t = singles.tile([128, 3 * H], BF, tag=f"whht{c}")
                nc.sync.dma_start(t[:], whht[c * 128:(c + 1) * 128, :])
                whht_sb.append(t)
            projwt_sb = []
            for c in range(4):
                t = singles.tile([128, E], BF, tag=f"projwt{c}")
                nc.sync.dma_start(t[:], projwt[c * 128:(c + 1) * 128, :])
                projwt_sb.append(t)
            bhhn_sb = singles.tile([1, H], BF, tag="bhhn")
            nc.sync.dma_start(bhhn_sb[:], bhhn[:])
            ones1 = singles.tile([1, 128], BF, tag="ones1")
            nc.vector.memset(ones1[:], 1.0)
            ident = singles.tile([128, 128], BF, tag="ident")
            make_identity(nc, ident[:])
            hzero = singles.tile([128, H], BF, tag="hzero")
            nc.vector.memset(hzero[:], 0.0)

            emb_ps = psE.tile([128, E], F32, tag="emb")

            h_prev = hzero
            hT_prev = None
            pairT = None
            quadT = None
            nproj = 0
            for l in range(steps):
                gx = gxp.tile([128, 1, 3 * H], BF, tag="gx")
                nc.gpsimd.dma_gather(
                    gx[:], table[:], idx_sb[:, l * 8:(l + 1) * 8], 128, 128,
                    elem_size=3 * H,
                )
                gxr = gx[:, 0, 0:H]
                gxz = gx[:, 0, H:2 * H]
                gxn = gx[:, 0, 2 * H:3 * H]

                # gate matmuls, gate-major (ps_r completes first); gx_r/gx_z
                # are PSUM-injected via exact identity matmuls
                ps_r = psg.tile([128, H], F32, tag="psr")
                ps_z = psg.tile([128, H], F32, tag="psz")
                ps_n = psg.tile([128, H], F32, tag="psn")
                nc.tensor.matmul(ps_r[:], ident[:], gxr, start=True,
                                 stop=(l == 0))
                if l > 0:
                    for c in range(4):
                        nc.tensor.matmul(ps_r[:], hT_prev[:, c, :],
                                         whht_sb[c][:, 0:H],
                                         start=False, stop=(c == 3))
                nc.tensor.matmul(ps_n[:], ones1[:], bhhn_sb[:], start=True,
                                 stop=(l == 0))
                if l > 0:
                    for c in range(4):
                        nc.tensor.matmul(ps_n[:], hT_prev[:, c, :],
                                         whht_sb[c][:, 2 * H:3 * H],
                                         start=False, stop=(c == 3))
                nc.tensor.matmul(ps_z[:], ident[:], gxz, start=True,
                                 stop=(l == 0))
                if l > 0:
                    for c in range(4):
                        nc.tensor.matmul(ps_z[:], hT_prev[:, c, :],
                                         whht_sb[c][:, H:2 * H],
                                         start=False, stop=(c == 3))

                # gates + update + transpose, chunked along H
                r = gatep.tile([128, H], BF, tag="r")
                z = gatep.tile([128, H], BF, tag="z")
                h_new = statep.tile([128, H], BF, tag="h")
                psT_t = psT.tile([128, 4, 128], BF, tag="ht")
                hT_new = statep.tile([128, 4, 128], BF, tag="hT")
                ndve = 0
                for off, tsz, cs in chunks:
                    sl = slice(off, off + tsz)
                    nc.scalar.activation(r[:, sl], ps_r[:, sl], SIG)
                    nc.scalar.activation(z[:, sl], ps_z[:, sl], SIG)
                    p = gatep.tile([128, tsz], BF, tag=f"p{off}")
                    nc.vector.tensor_mul(p[:], r[:, sl], ps_n[:, sl])
                    t_n = gatep.tile([128, tsz], BF, tag=f"tn{off}")
                    nc.vector.tensor_add(t_n[:], p[:], gxn[:, sl])
                    n_g = gatep.tile([128, tsz], BF, tag=f"ng{off}")
                    nc.scalar.activation(n_g[:], t_n[:], TANH)
                    # update h = (1-z)*n + z*h; z*h and (1-z) are emitted
                    # after tanh so DVE computes them during the tanh wait
                    u = gatep.tile([128, tsz], BF, tag=f"u{off}")
                    nc.vector.tensor_scalar(u[:], z[:, sl], -1.0, 1.0,
                                            mybir.AluOpType.mult,
                                            mybir.AluOpType.add)
                    w = gatep.tile([128, tsz], BF, tag=f"w{off}")
                    nc.vector.tensor_mul(w[:], z[:, sl], h_prev[:, sl])
                    vv = gatep.tile([128, tsz], BF, tag=f"v{off}")
                    nc.vector.tensor_mul(vv[:], u[:], n_g[:])
                    nc.vector.tensor_add(h_new[:, sl], vv[:], w[:])
                    for c in cs:
                        nc.tensor.transpose(psT_t[:, c, :],
                                            h_new[:, c * 128:(c + 1) * 128],
                                            ident[:])
                        if ndve < ht_copy_dve:
                            nc.vector.tensor_copy(hT_new[:, c, :], psT_t[:, c, :])
                            ndve += 1
                        else:
                            nc.scalar.copy(hT_new[:, c, :], psT_t[:, c, :])

                # pooling: bf16 pair/quad partial sums (in transposed layout),
                # projected into the fp32 PSUM accumulator once per 4 steps.
                # Priority-demoted so it fills scheduling gaps.
                _p0 = tc.cur_priority
                tc.cur_priority = _p0 + demote_pool
                if l % 2 == 0:
                    pairT = hT_new
                else:
                    npair = statep.tile([128, 4, 128], BF, tag="pairT")
                    nc.vector.tensor_add(npair[:], pairT[:], hT_new[:])
                    if l % 4 == 1:
                        quadT = npair
                    else:
                        nquad = statep.tile([128, 4, 128], BF, tag="quadT")
                        nc.vector.tensor_add(nquad[:], quadT[:], npair[:])
                        for c in range(4):
                            nc.tensor.matmul(emb_ps[:], nquad[:, c, :],
                                             projwt_sb[c][:],
                                             start=(nproj == 0),
                                             stop=(l == steps - 1 and c == 3))
                            nproj += 1
                tc.cur_priority = _p0

                h_prev = h_new
                hT_prev = hT_new

            zsb = singles.tile([128, E], F32, tag="zout")
            nc.scalar.copy(zsb[:], emb_ps[:])
            nc.sync.dma_start(zraw[:], zsb[:])

    nc.compile()
    return nc


def _build_phase2():
    nc = bacc.Bacc("TRN2", target_bir_lowering=False, debug=False)
    znt = nc.dram_tensor("znt", [2, 128, N], F32, kind="ExternalInput").ap()
    zntl = nc.dram_tensor("zntl", [2, 128, 128], F32, kind="ExternalInput").ap()
    posm = nc.dram_tensor("posm", [128, N], F32, kind="ExternalInput").ap()
    nd = nc.dram_tensor("nd", [128, 2], F32, kind="ExternalOutput").ap()

    with tile.TileContext(nc) as tc:
        with (
            tc.tile_pool(name="sb", bufs=1) as sb,
            tc.tile_pool(name="ps", bufs=2, space="PSUM") as ps,
        ):
            znt_sb = []
            for c in range(2):
                t = sb.tile([128, N], F32, tag=f"znt{c}")
                nc.sync.dma_start(t[:], znt[c, :, :])
                znt_sb.append(t)
            zntl_sb = []
            for c in range(2):
                t = sb.tile([128, 128], F32, tag=f"zntl{c}")
                nc.sync.dma_start(t[:], zntl[c, :, :])
                zntl_sb.append(t)
            posm_sb = sb.tile([128, N], F32, tag="posm")
            nc.sync.dma_start(posm_sb[:], posm[:])

            s_parts, n_parts = [], []
            junk = sb.tile([128, 512], F32, tag="junk")
            for half in range(2):
                pst = ps.tile([128, 512], F32, tag="sim")
                for c in range(2):
                    nc.tensor.matmul(pst[:], zntl_sb[c][:],
                                     znt_sb[c][:, half * 512:(half + 1) * 512],
                                     start=(c == 0), stop=(c == 1))
                es = sb.tile([128, 512], F32, tag=f"es{half}")
                s_p = sb.tile([128, 1], F32, tag=f"sp{half}")
                nc.scalar.activation(es[:], pst[:], EXP,
                                     scale=1.0 / TAU, accum_out=s_p[:])
                n_p = sb.tile([128, 1], F32, tag=f"np{half}")
                nc.vector.scalar_tensor_tensor(
                    junk[:], es[:], 1.0, posm_sb[:, half * 512:(half + 1) * 512],
                    op0=mybir.AluOpType.mult, op1=mybir.AluOpType.mult,
                    accum_out=n_p[:])
                s_parts.append(s_p)
                n_parts.append(n_p)

            out_sb = sb.tile([128, 2], F32, tag="out")
            nc.vector.tensor_add(out_sb[:, 0:1], n_parts[0][:], n_parts[1][:])
            nc.vector.tensor_add(out_sb[:, 1:2], s_parts[0][:], s_parts[1][:])
            nc.sync.dma_start(nd[:], out_sb[:])

    nc.compile()
    return nc


_CACHE = {}

# Filled by kernel() on every call: [("phase1", BassKernelResults), ...].
# exec_time_ns is populated when the KERNEL_PROFILE env var is set.
LAST_RESULTS = []


def _get_programs():
    if "nc1" not in _CACHE:
        _CACHE["nc1"] = _build_phase1()
        _CACHE["nc2"] = _build_phase2()
    return _CACHE["nc1"], _CACHE["nc2"]


def _run(nc, in_maps, name):
    kw = {}
    if os.environ.get("KERNEL_PROFILE"):
        kw = dict(trace=True)
        d = os.environ.get("KERNEL_PROFILE_DIR")
        if d:
            kw["tmpdir"] = os.path.join(d, name)
            os.makedirs(kw["tmpdir"], exist_ok=True)
    res = run_bass_kernel_spmd(nc, in_maps, core_ids=list(range(NCORES)), **kw)
    LAST_RESULTS.append((name, res))
    return res


def _make_idx(tok):
    """tok: [128, L] token ids for one core -> [128, L*8] int16 index tile
    in the wrap-by-16 layout dma_gather expects (replicated to 128 rows)."""
    steps = tok.shape[1]
    a = (tok.astype(np.int16).reshape(8, 16, steps).transpose(1, 2, 0)
         .reshape(16, steps * 8))
    return np.ascontiguousarray(np.tile(a, (8, 1)))


def kernel(sequence, labels, starts, W_ih, W_hh, b_ih, b_hh, proj_W, proj_b):
    bf16 = ml_dtypes.bfloat16
    sequence = np.asarray(sequence)
    labels = np.asarray(labels)
    starts = np.asarray(starts)
    W_ih = np.asarray(W_ih, np.float32)
    W_hh = np.asarray(W_hh, np.float32)
    b_ih = np.asarray(b_ih, np.float32)
    b_hh = np.asarray(b_hh, np.float32)
    proj_W = np.asarray(proj_W, np.float32)
    proj_b = np.asarray(proj_b, np.float32)

    nc1, nc2 = _get_programs()

    # ---- host: subsequence extraction + sharding (pure indexing) ----
    idx = starts[:, :, None].astype(np.int64) + np.arange(L)[None, None, :]
    sub = sequence[np.arange(B)[None, :, None], idx].reshape(N, L)
    lab = np.tile(labels, K)

    bcomb = np.concatenate([
        b_ih[:H] + b_hh[:H], b_ih[H:2 * H] + b_hh[H:2 * H], b_ih[2 * H:]
    ]).astype(np.float32)
    shared = dict(
        table=np.ascontiguousarray((W_ih.T + bcomb[None, :]).astype(bf16)),
        whht=np.ascontiguousarray(W_hh.T.astype(bf16)),
        bhhn=np.ascontiguousarray(b_hh[2 * H:].reshape(1, H).astype(bf16)),
        projwt=np.ascontiguousarray((proj_W.T / L).astype(bf16)),
    )
    in_maps1 = []
    for c in range(NCORES):
        m = dict(shared)
        m["idx"] = _make_idx(sub[c * NLOC:(c + 1) * NLOC, :])
        in_maps1.append(m)

    LAST_RESULTS.clear()
    res1 = _run(nc1, in_maps1, "phase1")
    z = np.concatenate([res1.results[c]["zraw"] for c in range(NCORES)], 0)
    z = z + proj_b[None, :]

    # ---- host glue: normalize + phase-2 operands ----
    norm = np.maximum(np.sqrt((z ** 2).sum(1, keepdims=True)), 1e-12)
    zn = (z / norm).astype(np.float32)
    znt_in = np.ascontiguousarray(zn.T).reshape(2, 128, N)
    pos = (lab[None, :] == lab[:, None]) & ~np.eye(N, dtype=bool)
    posf = pos.astype(np.float32)

    in_maps2 = []
    for c in range(NCORES):
        in_maps2.append(dict(
            znt=znt_in,
            zntl=np.ascontiguousarray(znt_in[:, :, c * NLOC:(c + 1) * NLOC]),
            posm=posf[c * NLOC:(c + 1) * NLOC, :],
        ))
    res2 = _run(nc2, in_maps2, "phase2")
    nd = np.concatenate([res2.results[c]["nd"] for c in range(NCORES)], 0)

    num = nd[:, 0].astype(np.float64)
    ssum = nd[:, 1].astype(np.float64)
    es_ii = np.exp((zn.astype(np.float64) ** 2).sum(1) / TAU)
    den = ssum - es_ii
    has_pos = pos.any(1)
    li = -np.log(num / (den + 1e-10) + 1e-10)
    loss = np.where(has_pos, li, 0.0).sum() / max(int(has_pos.sum()), 1)
    return np.float32(loss)


# revision 7
# speedup vs baseline: 1.0156x; 1.0156x over previous
"""Trainium2 Bass kernel for the CoLES problem (GRU encoder + NT-Xent loss).

Strategy (8 NeuronCores, data-parallel over the K*B=1024 subsequences):
  * host: extract subsequence token ids (pure indexing); shard 128 rows/core.
  * phase 1 (per core): the one-hot input GEMM is an embedding gather from
    (W_ih.T + baked biases) via dma_gather (bf16 table in HBM, prefetched
    6 steps deep); 64 GRU steps with bf16 matmuls accumulated in fp32 PSUM.
    gx_r / gx_z are injected into PSUM with identity matmuls so the r/z
    gates are a single ACT sigmoid straight from PSUM; the gh matmuls are
    emitted gate-major so ps_r completes first; the n-gate/update/transpose
    tail is chunked 256+256 along H so next-step matmuls overlap the tail.
    Mean-pool+projection is folded into PSUM-accumulated matmuls of bf16
    pair/quad partial sums against proj_W.T/64 (once per 4 steps).
  * host glue: add proj_b, L2-normalize embeddings, build phase-2 operands.
  * phase 2 (per core): a 128x1024 block of the similarity matrix (fp32
    matmul), es=exp(sim/tau) with fused row-sum, fused row-sum of es*pos.
  * host: loss = mean(-log(num/(den+1e-10)+1e-10)) with has_pos handling.
"""
import os
import sys

sys.path.insert(0, "/opt/trn_rl_repo")

import numpy as np
import ml_dtypes

import concourse.bass as bass
import concourse.tile as tile
from concourse import bacc, mybir
from concourse.masks import make_identity
from concourse.bass_utils import run_bass_kernel_spmd

BF = mybir.dt.bfloat16
F32 = mybir.dt.float32
I16 = mybir.dt.int16

B, S, V, H, E, L, K = 512, 512, 1024, 512, 256, 64, 2
TAU = 0.1
NCORES = 8
N = K * B
NLOC = N // NCORES  # 128

SIG = mybir.ActivationFunctionType.Sigmoid
TANH = mybir.ActivationFunctionType.Tanh
EXP = mybir.ActivationFunctionType.Exp


def _build_phase1(steps=L, gx_bufs=8, tail_plan=(256, 256), ht_copy_dve=4,
                  demote_pool=100000):
    nc = bacc.Bacc("TRN2", target_bir_lowering=False, debug=False)
    table = nc.dram_tensor("table", [V, 3 * H], BF, kind="ExternalInput").ap()
    whht = nc.dram_tensor("whht", [H, 3 * H], BF, kind="ExternalInput").ap()
    bhhn = nc.dram_tensor("bhhn", [1, H], BF, kind="ExternalInput").ap()
    projwt = nc.dram_tensor("projwt", [H, E], BF, kind="ExternalInput").ap()
    idx = nc.dram_tensor("idx", [128, steps * 8], I16, kind="ExternalInput").ap()
    zraw = nc.dram_tensor("zraw", [NLOC, E], F32, kind="ExternalOutput").ap()

    chunks = []
    off = 0
    for t in tail_plan:
        chunks.append((off, t, list(range(off // 128, (off + t) // 128))))
        off += t

    with tile.TileContext(nc) as tc:
        with (
            tc.tile_pool(name="singles", bufs=1) as singles,
            tc.tile_pool(name="gx", bufs=gx_bufs) as gxp,
            tc.tile_pool(name="state", bufs=2) as statep,
            tc.tile_pool(name="gates", bufs=2) as gatep,
            tc.tile_pool(name="psg", bufs=2, space="PSUM") as psg,
            tc.tile_pool(name="psT", bufs=1, space="PSUM") as psT,
            tc.tile_pool(name="psE", bufs=1, space="PSUM") as psE,
        ):
            idx_sb = singles.tile([128, steps * 8], I16, tag="idx")
            nc.sync.dma_start(idx_sb[:], idx[:])
            whht_sb = []
            for c in range(4):
                t = singles.tile([128, 3 * H], BF, tag=f"whht{c}")
                nc.sync.dma_start(t[:], whht[c * 128:(c + 1) * 128, :])
                whht_sb.append(t)
            projwt_sb = []
            for c in range(4):
                t = singles.tile([128, E], BF, tag=f"projwt{c}")
                nc.sync.dma_start(t[:], projwt[c * 128:(c + 1) * 128, :])
                projwt_sb.append(t)
            bhhn_sb = singles.tile([1, H], BF, tag="bhhn")
            nc.sync.dma_start(bhhn_sb[:], bhhn[:])
            ones1 = singles.tile([1, 128], BF, tag="ones1")
            nc.vector.memset(ones1[:], 1.0)
            ident = singles.tile([128, 128], BF, tag="ident")
            make_identity(nc, ident[:])
            hzero = singles.tile([128, H], BF, tag="hzero")
            nc.vector.memset(hzero[:], 0.0)

            emb_ps = psE.tile([128, E], F32, tag="emb")

            h_prev = hzero
            hT_prev = None
            pairT = None
            quadT = None
            nproj = 0
            for l in range(steps):
                gx = gxp.tile([128, 1, 3 * H], BF, tag="gx")
                nc.gpsimd.dma_gather(
                    gx[:], table[:], idx_sb[:, l * 8:(l + 1) * 8], 128, 128,
                    elem_size=3 * H,
                )
                gxr = gx[:, 0, 0:H]
                gxz = gx[:, 0, H:2 * H]
                gxn = gx[:, 0, 2 * H:3 * H]

                # gate matmuls, gate-major (ps_r completes first); gx_r/gx_z
                # are PSUM-injected via exact identity matmuls
                ps_r = psg.tile([128, H], F32, tag="psr")
                ps_z = psg.tile([128, H], F32, tag="psz")
                ps_n = psg.tile([128, H], F32, tag="psn")
                nc.tensor.matmul(ps_r[:], ident[:], gxr, start=True,
                                 stop=(l == 0))
                if l > 0:
                    for c in range(4):
                        nc.tensor.matmul(ps_r[:], hT_prev[:, c, :],
                                         whht_sb[c][:, 0:H],
                                         start=False, stop=(c == 3))
                nc.tensor.matmul(ps_n[:], ones1[:], bhhn_sb[:], start=True,
                                 stop=(l == 0))
                if l > 0:
                    for c in range(4):
                        nc.tensor.matmul(ps_n[:], hT_prev[:, c, :],
                                         whht_sb[c][:, 2 * H:3 * H],
                                         start=False, stop=(c == 3))
                nc.tensor.matmul(ps_z[:], ident[:], gxz, start=True,
                                 stop=(l == 0))
                if l > 0:
                    for c in range(4):
                        nc.tensor.matmul(ps_z[:], hT_prev[:, c, :],
                                         whht_sb[c][:, H:2 * H],
                                         start=False, stop=(c == 3))

                # gates + update + transpose, chunked along H
                r = gatep.tile([128, H], BF, tag="r")
                z = gatep.tile([128, H], BF, tag="z")
                h_new = statep.tile([128, H], BF, tag="h")
                psT_t = psT.tile([128, 4, 128], BF, tag="ht")
                hT_new = statep.tile([128, 4, 128], BF, tag="hT")
                ndve = 0
                for off, tsz, cs in chunks:
                    sl = slice(off, off + tsz)
                    nc.scalar.activation(r[:, sl], ps_r[:, sl], SIG)
                    nc.scalar.activation(z[:, sl], ps_z[:, sl], SIG)
                    p = gatep.tile([128, tsz], BF, tag=f"p{off}")
                    nc.vector.tensor_mul(p[:], r[:, sl], ps_n[:, sl])
                    t_n = gatep.tile([128, tsz], BF, tag=f"tn{off}")
                    nc.vector.tensor_add(t_n[:], p[:], gxn[:, sl])
                    n_g = gatep.tile([128, tsz], BF, tag=f"ng{off}")
                    nc.scalar.activation(n_g[:], t_n[:], TANH)
                    # update h = (1-z)*n + z*h; z*h and (1-z) are emitted
                    # after tanh so DVE computes them during the tanh wait
                    u = gatep.tile([128, tsz], BF, tag=f"u{off}")
                    nc.vector.tensor_scalar(u[:], z[:, sl], -1.0, 1.0,
                                            mybir.AluOpType.mult,
                                            mybir.AluOpType.add)
                    w = gatep.tile([128, tsz], BF, tag=f"w{off}")
                    nc.vector.tensor_mul(w[:], z[:, sl], h_prev[:, sl])
                    vv = gatep.tile([128, tsz], BF, tag=f"v{off}")
                    nc.vector.tensor_mul(vv[:], u[:], n_g[:])
                    nc.vector.tensor_add(h_new[:, sl], vv[:], w[:])
                    for c in cs:
                        nc.tensor.transpose(psT_t[:, c, :],
                                            h_new[:, c * 128:(c + 1) * 128],
                                            ident[:])
                        if ndve < ht_copy_dve:
                            nc.vector.tensor_copy(hT_new[:, c, :], psT_t[:, c, :])
                            ndve += 1
                        else:
                            nc.scalar.copy(hT_new[:, c, :], psT_t[:, c, :])

                # pooling: bf16 pair/quad partial sums (in transposed layout),
                # projected into the fp32 PSUM accumulator once per 4 steps.
                # Priority-demoted so it fills scheduling gaps.
                _p0 = tc.cur_priority
                tc.cur_priority = _p0 + demote_pool
                if l % 2 == 0:
                    pairT = hT_new
                else:
                    npair = statep.tile([128, 4, 128], BF, tag="pairT")
                    nc.vector.tensor_add(npair[:], pairT[:], hT_new[:])
                    if l % 4 == 1:
                        quadT = npair
                    else:
                        nquad = statep.tile([128, 4, 128], BF, tag="quadT")
                        nc.vector.tensor_add(nquad[:], quadT[:], npair[:])
                        for c in range(4):
                            nc.tensor.matmul(emb_ps[:], nquad[:, c, :],
                                             projwt_sb[c][:],
                                             start=(nproj == 0),
                                             stop=(l == steps - 1 and c == 3))
                            nproj += 1
                tc.cur_priority = _p0

                h_prev = h_new
                hT_prev = hT_new

            zsb = singles.tile([128, E], F32, tag="zout")
            nc.scalar.copy(zsb[:], emb_ps[:])
            nc.sync.dma_start(zraw[:], zsb[:])

    nc.compile()
    return nc


def _build_phase2():
    nc = bacc.Bacc("TRN2", target_bir_lowering=False, debug=False)
    znt = nc.dram_tensor("znt", [2, 128, N], BF, kind="ExternalInput").ap()
    zntl = nc.dram_tensor("zntl", [2, 128, 128], BF, kind="ExternalInput").ap()
    posm = nc.dram_tensor("posm", [128, N], BF, kind="ExternalInput").ap()
    nd = nc.dram_tensor("nd", [128, 2], F32, kind="ExternalOutput").ap()

    with tile.TileContext(nc) as tc:
        with (
            tc.tile_pool(name="sb", bufs=1) as sb,
            tc.tile_pool(name="ps", bufs=2, space="PSUM") as ps,
        ):
            junk = sb.tile([128, 512], F32, tag="junk")
            # warm the exp activation table while the input DMAs run
            warm = sb.tile([128, 8], F32, tag="warm")
            nc.vector.memset(warm[:], 0.0)
            nc.scalar.activation(warm[:], warm[:], EXP)
            zntl_sb = []
            for c in range(2):
                t = sb.tile([128, 128], BF, tag=f"zntl{c}")
                nc.sync.dma_start(t[:], zntl[c, :, :])
                zntl_sb.append(t)
            znt_sb = []
            for c in range(2):
                full = sb.tile([128, N], BF, tag=f"znt{c}")
                for hf in range(2):
                    nc.sync.dma_start(full[:, hf * 512:(hf + 1) * 512],
                                      znt[c, :, hf * 512:(hf + 1) * 512])
                znt_sb.append(full)
            posm_sb = sb.tile([128, N], BF, tag="posm")
            nc.sync.dma_start(posm_sb[:], posm[:])

            s_parts, n_parts = [], []
            for half in range(2):
                pst = ps.tile([128, 512], F32, tag="sim")
                for c in range(2):
                    nc.tensor.matmul(pst[:], zntl_sb[c][:],
                                     znt_sb[c][:, half * 512:(half + 1) * 512],
                                     start=(c == 0), stop=(c == 1))
                es = sb.tile([128, 512], F32, tag=f"es{half}")
                s_p = sb.tile([128, 1], F32, tag=f"sp{half}")
                nc.scalar.activation(es[:], pst[:], EXP,
                                     scale=1.0 / TAU, accum_out=s_p[:])
                n_p = sb.tile([128, 1], F32, tag=f"np{half}")
                nc.vector.scalar_tensor_tensor(
                    junk[:], es[:], 1.0, posm_sb[:, half * 512:(half + 1) * 512],
                    op0=mybir.AluOpType.mult, op1=mybir.AluOpType.mult,
                    accum_out=n_p[:])
                s_parts.append(s_p)
                n_parts.append(n_p)

            out_sb = sb.tile([128, 2], F32, tag="out")
            nc.vector.tensor_add(out_sb[:, 0:1], n_parts[0][:], n_parts[1][:])
            nc.vector.tensor_add(out_sb[:, 1:2], s_parts[0][:], s_parts[1][:])
            nc.sync.dma_start(nd[:], out_sb[:])

    nc.compile()
    return nc


_CACHE = {}

# Filled by kernel() on every call: [("phase1", BassKernelResults), ...].
# exec_time_ns is populated when the KERNEL_PROFILE env var is set.
LAST_RESULTS = []


def _get_programs():
    if "nc1" not in _CACHE:
        _CACHE["nc1"] = _build_phase1()
        _CACHE["nc2"] = _build_phase2()
    return _CACHE["nc1"], _CACHE["nc2"]


def _run(nc, in_maps, name):
    kw = {}
    if os.environ.get("KERNEL_PROFILE"):
        kw = dict(trace=True)
        d = os.environ.get("KERNEL_PROFILE_DIR")
        if d:
            kw["tmpdir"] = os.path.join(d, name)
            os.makedirs(kw["tmpdir"], exist_ok=True)
    res = run_bass_kernel_spmd(nc, in_maps, core_ids=list(range(NCORES)), **kw)
    LAST_RESULTS.append((name, res))
    return res


def _make_idx(tok):
    """tok: [128, L] token ids for one core -> [128, L*8] int16 index tile
    in the wrap-by-16 layout dma_gather expects (replicated to 128 rows)."""
    steps = tok.shape[1]
    a = (tok.astype(np.int16).reshape(8, 16, steps).transpose(1, 2, 0)
         .reshape(16, steps * 8))
    return np.ascontiguousarray(np.tile(a, (8, 1)))


def kernel(sequence, labels, starts, W_ih, W_hh, b_ih, b_hh, proj_W, proj_b):
    bf16 = ml_dtypes.bfloat16
    sequence = np.asarray(sequence)
    labels = np.asarray(labels)
    starts = np.asarray(starts)
    W_ih = np.asarray(W_ih, np.float32)
    W_hh = np.asarray(W_hh, np.float32)
    b_ih = np.asarray(b_ih, np.float32)
    b_hh = np.asarray(b_hh, np.float32)
    proj_W = np.asarray(proj_W, np.float32)
    proj_b = np.asarray(proj_b, np.float32)

    nc1, nc2 = _get_programs()

    # ---- host: subsequence extraction + sharding (pure indexing) ----
    idx = starts[:, :, None].astype(np.int64) + np.arange(L)[None, None, :]
    sub = sequence[np.arange(B)[None, :, None], idx].reshape(N, L)
    lab = np.tile(labels, K)

    bcomb = np.concatenate([
        b_ih[:H] + b_hh[:H], b_ih[H:2 * H] + b_hh[H:2 * H], b_ih[2 * H:]
    ]).astype(np.float32)
    shared = dict(
        table=np.ascontiguousarray((W_ih.T + bcomb[None, :]).astype(bf16)),
        whht=np.ascontiguousarray(W_hh.T.astype(bf16)),
        bhhn=np.ascontiguousarray(b_hh[2 * H:].reshape(1, H).astype(bf16)),
        projwt=np.ascontiguousarray((proj_W.T / L).astype(bf16)),
    )
    in_maps1 = []
    for c in range(NCORES):
        m = dict(shared)
        m["idx"] = _make_idx(sub[c * NLOC:(c + 1) * NLOC, :])
        in_maps1.append(m)

    LAST_RESULTS.clear()
    res1 = _run(nc1, in_maps1, "phase1")
    z = np.concatenate([res1.results[c]["zraw"] for c in range(NCORES)], 0)
    z = z + proj_b[None, :]

    # ---- host glue: normalize + phase-2 operands ----
    norm = np.maximum(np.sqrt((z ** 2).sum(1, keepdims=True)), 1e-12)
    zn = (z / norm).astype(np.float32)
    znb = zn.astype(bf16)
    znt_in = np.ascontiguousarray(znb.T).reshape(2, 128, N)
    pos = (lab[None, :] == lab[:, None]) & ~np.eye(N, dtype=bool)
    posf = pos.astype(bf16)

    in_maps2 = []
    for c in range(NCORES):
        in_maps2.append(dict(
            znt=znt_in,
            zntl=np.ascontiguousarray(znt_in[:, :, c * NLOC:(c + 1) * NLOC]),
            posm=posf[c * NLOC:(c + 1) * NLOC, :],
        ))
    res2 = _run(nc2, in_maps2, "phase2")
    nd = np.concatenate([res2.results[c]["nd"] for c in range(NCORES)], 0)

    num = nd[:, 0].astype(np.float64)
    ssum = nd[:, 1].astype(np.float64)
    zb64 = znb.astype(np.float64)
    es_ii = np.exp((zb64 * zb64).sum(1) / TAU)
    den = ssum - es_ii
    has_pos = pos.any(1)
    li = -np.log(num / (den + 1e-10) + 1e-10)
    loss = np.where(has_pos, li, 0.0).sum() / max(int(has_pos.sum()), 1)
    return np.float32(loss)


# revision 9
# speedup vs baseline: 1.0179x; 1.0022x over previous
"""Trainium2 Bass kernel for the CoLES problem (GRU encoder + NT-Xent loss).

Strategy (8 NeuronCores, data-parallel over the K*B=1024 subsequences):
  * host: extract subsequence token ids (pure indexing); shard 128 rows/core.
  * phase 1 (per core): the one-hot input GEMM is an embedding gather from
    (W_ih.T + baked biases) via dma_gather (bf16 table in HBM, prefetched
    6 steps deep); 64 GRU steps with bf16 matmuls accumulated in fp32 PSUM.
    gx_r / gx_z are injected into PSUM with identity matmuls so the r/z
    gates are a single ACT sigmoid straight from PSUM; the gh matmuls are
    emitted gate-major so ps_r completes first; the n-gate/update/transpose
    tail is chunked 256+256 along H so next-step matmuls overlap the tail.
    Mean-pool+projection is folded into PSUM-accumulated matmuls of bf16
    pair/quad partial sums against proj_W.T/64 (once per 4 steps).
  * host glue: add proj_b, L2-normalize embeddings, build phase-2 operands.
  * phase 2 (per core): a 128x1024 block of the similarity matrix (bf16
    matmul, fp32 PSUM; host es_ii correction uses the same bf16-rounded zn),
    es=exp(sim/tau) with fused row-sum, fused row-sum of es*pos; the exp
    activation table is pre-warmed under the input DMAs.
  * host: loss = mean(-log(num/(den+1e-10)+1e-10)) with has_pos handling.
"""
import os
import sys

sys.path.insert(0, "/opt/trn_rl_repo")

import numpy as np
import ml_dtypes

import concourse.bass as bass
import concourse.tile as tile
from concourse import bacc, mybir
from concourse.masks import make_identity
from concourse.bass_utils import run_bass_kernel_spmd

BF = mybir.dt.bfloat16
F32 = mybir.dt.float32
I16 = mybir.dt.int16

B, S, V, H, E, L, K = 512, 512, 1024, 512, 256, 64, 2
TAU = 0.1
NCORES = 8
N = K * B
NLOC = N // NCORES  # 128

SIG = mybir.ActivationFunctionType.Sigmoid
TANH = mybir.ActivationFunctionType.Tanh
EXP = mybir.ActivationFunctionType.Exp


def _build_phase1(steps=L, gx_bufs=8, tail_plan=(256, 256), ht_copy_dve=4,
                  demote_pool=100000):
    nc = bacc.Bacc("TRN2", target_bir_lowering=False, debug=False)
    table = nc.dram_tensor("table", [V, 3 * H], BF, kind="ExternalInput").ap()
    whht = nc.dram_tensor("whht", [H, 3 * H], BF, kind="ExternalInput").ap()
    bhhn = nc.dram_tensor("bhhn", [1, H], BF, kind="ExternalInput").ap()
    projwt = nc.dram_tensor("projwt", [H, E], BF, kind="ExternalInput").ap()
    idx = nc.dram_tensor("idx", [128, steps * 8], I16, kind="ExternalInput").ap()
    zraw = nc.dram_tensor("zraw", [NLOC, E], F32, kind="ExternalOutput").ap()

    chunks = []
    off = 0
    for t in tail_plan:
        chunks.append((off, t, list(range(off // 128, (off + t) // 128))))
        off += t

    with tile.TileContext(nc) as tc:
        with (
            tc.tile_pool(name="singles", bufs=1) as singles,
            tc.tile_pool(name="gx", bufs=gx_bufs) as gxp,
            tc.tile_pool(name="state", bufs=2) as statep,
            tc.tile_pool(name="gates", bufs=2) as gatep,
            tc.tile_pool(name="psg", bufs=2, space="PSUM") as psg,
            tc.tile_pool(name="psT", bufs=1, space="PSUM") as psT,
            tc.tile_pool(name="psE", bufs=1, space="PSUM") as psE,
        ):
            idx_sb = singles.tile([128, steps * 8], I16, tag="idx")
            nc.sync.dma_start(idx_sb[:], idx[:])
            whht_sb = []
            for c in range(4):
                t = singles.tile([128, 3 * H], BF, tag=f"whht{c}")
                nc.sync.dma_start(t[:], whht[c * 128:(c + 1) * 128, :])
                whht_sb.append(t)
            projwt_sb = []
            for c in range(4):
                t = singles.tile([128, E], BF, tag=f"projwt{c}")
                nc.sync.dma_start(t[:], projwt[c * 128:(c + 1) * 128, :])
                projwt_sb.append(t)
            bhhn_sb = singles.tile([1, H], BF, tag="bhhn")
            nc.sync.dma_start(bhhn_sb[:], bhhn[:])
            ones1 = singles.tile([1, 128], BF, tag="ones1")
            nc.vector.memset(ones1[:], 1.0)
            ident = singles.tile([128, 128], BF, tag="ident")
            make_identity(nc, ident[:])
            hzero = singles.tile([128, H], BF, tag="hzero")
            nc.vector.memset(hzero[:], 0.0)

            emb_ps = psE.tile([128, E], F32, tag="emb")

            h_prev = hzero
            hT_prev = None
            pairT = None
            quadT = None
            nproj = 0
            for l in range(steps):
                gx = gxp.tile([128, 1, 3 * H], BF, tag="gx")
                nc.gpsimd.dma_gather(
                    gx[:], table[:], idx_sb[:, l * 8:(l + 1) * 8], 128, 128,
                    elem_size=3 * H,
                )
                gxr = gx[:, 0, 0:H]
                gxz = gx[:, 0, H:2 * H]
                gxn = gx[:, 0, 2 * H:3 * H]

                # gate matmuls, gate-major (ps_r completes first); gx_r/gx_z
                # are PSUM-injected via exact identity matmuls
                ps_r = psg.tile([128, H], F32, tag="psr")
                ps_z = psg.tile([128, H], F32, tag="psz")
                ps_n = psg.tile([128, H], F32, tag="psn")
                nc.tensor.matmul(ps_r[:], ident[:], gxr, start=True,
                                 stop=(l == 0))
                if l > 0:
                    for c in range(4):
                        nc.tensor.matmul(ps_r[:], hT_prev[:, c, :],
                                         whht_sb[c][:, 0:H],
                                         start=False, stop=(c == 3))
                nc.tensor.matmul(ps_n[:], ones1[:], bhhn_sb[:], start=True,
                                 stop=(l == 0))
                if l > 0:
                    for c in range(4):
                        nc.tensor.matmul(ps_n[:], hT_prev[:, c, :],
                                         whht_sb[c][:, 2 * H:3 * H],
                                         start=False, stop=(c == 3))
                nc.tensor.matmul(ps_z[:], ident[:], gxz, start=True,
                                 stop=(l == 0))
                if l > 0:
                    for c in range(4):
                        nc.tensor.matmul(ps_z[:], hT_prev[:, c, :],
                                         whht_sb[c][:, H:2 * H],
                                         start=False, stop=(c == 3))

                # gates + update + transpose, chunked along H
                r = gatep.tile([128, H], BF, tag="r")
                z = gatep.tile([128, H], BF, tag="z")
                h_new = statep.tile([128, H], BF, tag="h")
                psT_t = psT.tile([128, 4, 128], BF, tag="ht")
                hT_new = statep.tile([128, 4, 128], BF, tag="hT")
                ndve = 0
                for off, tsz, cs in chunks:
                    sl = slice(off, off + tsz)
                    nc.scalar.activation(r[:, sl], ps_r[:, sl], SIG)
                    nc.scalar.activation(z[:, sl], ps_z[:, sl], SIG)
                    p = gatep.tile([128, tsz], BF, tag=f"p{off}")
                    nc.vector.tensor_mul(p[:], r[:, sl], ps_n[:, sl])
                    t_n = gatep.tile([128, tsz], BF, tag=f"tn{off}")
                    nc.vector.tensor_add(t_n[:], p[:], gxn[:, sl])
                    n_g = gatep.tile([128, tsz], BF, tag=f"ng{off}")
                    nc.scalar.activation(n_g[:], t_n[:], TANH)
                    # update h = (1-z)*n + z*h; z*h and (1-z) are emitted
                    # after tanh so DVE computes them during the tanh wait
                    u = gatep.tile([128, tsz], BF, tag=f"u{off}")
                    nc.vector.tensor_scalar(u[:], z[:, sl], -1.0, 1.0,
                                            mybir.AluOpType.mult,
                                            mybir.AluOpType.add)
                    w = gatep.tile([128, tsz], BF, tag=f"w{off}")
                    nc.vector.tensor_mul(w[:], z[:, sl], h_prev[:, sl])
                    vv = gatep.tile([128, tsz], BF, tag=f"v{off}")
                    nc.vector.tensor_mul(vv[:], u[:], n_g[:])
                    nc.vector.tensor_add(h_new[:, sl], vv[:], w[:])
                    for c in cs:
                        nc.tensor.transpose(psT_t[:, c, :],
                                            h_new[:, c * 128:(c + 1) * 128],
                                            ident[:])
                        if ndve < ht_copy_dve:
                            nc.vector.tensor_copy(hT_new[:, c, :], psT_t[:, c, :])
                            ndve += 1
                        else:
                            nc.scalar.copy(hT_new[:, c, :], psT_t[:, c, :])

                # pooling: bf16 pair/quad partial sums (in transposed layout),
                # projected into the fp32 PSUM accumulator once per 4 steps.
                # Priority-demoted so it fills scheduling gaps.
                _p0 = tc.cur_priority
                tc.cur_priority = _p0 + demote_pool
                if l % 2 == 0:
                    pairT = hT_new
                else:
                    npair = statep.tile([128, 4, 128], BF, tag="pairT")
                    nc.vector.tensor_add(npair[:], pairT[:], hT_new[:])
                    if l % 4 == 1:
                        quadT = npair
                    else:
                        nquad = statep.tile([128, 4, 128], BF, tag="quadT")
                        nc.vector.tensor_add(nquad[:], quadT[:], npair[:])
                        for c in range(4):
                            nc.tensor.matmul(emb_ps[:], nquad[:, c, :],
                                             projwt_sb[c][:],
                                             start=(nproj == 0),
                                             stop=(l == steps - 1 and c == 3))
                            nproj += 1
                tc.cur_priority = _p0

                h_prev = h_new
                hT_prev = hT_new

            zsb = singles.tile([128, E], F32, tag="zout")
            nc.scalar.copy(zsb[:], emb_ps[:])
            nc.sync.dma_start(zraw[:], zsb[:])

    nc.compile()
    return nc


def _build_phase2():
    nc = bacc.Bacc("TRN2", target_bir_lowering=False, debug=False)
    znt = nc.dram_tensor("znt", [2, 128, N], BF, kind="ExternalInput").ap()
    zntl = nc.dram_tensor("zntl", [2, 128, 128], BF, kind="ExternalInput").ap()
    posm = nc.dram_tensor("posm", [128, N], BF, kind="ExternalInput").ap()
    nd = nc.dram_tensor("nd", [128, 2], F32, kind="ExternalOutput").ap()

    with tile.TileContext(nc) as tc:
        with (
            tc.tile_pool(name="sb", bufs=1) as sb,
            tc.tile_pool(name="ps", bufs=2, space="PSUM") as ps,
        ):
            junk = sb.tile([128, 512], F32, tag="junk")
            # warm the exp activation table while the input DMAs run
            warm = sb.tile([128, 8], F32, tag="warm")
            nc.vector.memset(warm[:], 0.0)
            nc.scalar.activation(warm[:], warm[:], EXP)
            # big znt transfers issued first so they stream earliest
            znt_sb = []
            for c in range(2):
                full = sb.tile([128, N], BF, tag=f"znt{c}")
                nc.sync.dma_start(full[:], znt[c, :, :])
                znt_sb.append(full)
            zntl_sb = []
            for c in range(2):
                t = sb.tile([128, 128], BF, tag=f"zntl{c}")
                nc.sync.dma_start(t[:], zntl[c, :, :])
                zntl_sb.append(t)
            posm_sb = sb.tile([128, N], BF, tag="posm")
            nc.sync.dma_start(posm_sb[:], posm[:])

            s_parts, n_parts = [], []
            for half in range(2):
                pst = ps.tile([128, 512], F32, tag="sim")
                for c in range(2):
                    nc.tensor.matmul(pst[:], zntl_sb[c][:],
                                     znt_sb[c][:, half * 512:(half + 1) * 512],
                                     start=(c == 0), stop=(c == 1))
                es = sb.tile([128, 512], F32, tag=f"es{half}")
                s_p = sb.tile([128, 1], F32, tag=f"sp{half}")
                nc.scalar.activation(es[:], pst[:], EXP,
                                     scale=1.0 / TAU, accum_out=s_p[:])
                n_p = sb.tile([128, 1], F32, tag=f"np{half}")
                nc.vector.scalar_tensor_tensor(
                    junk[:], es[:], 1.0, posm_sb[:, half * 512:(half + 1) * 512],
                    op0=mybir.AluOpType.mult, op1=mybir.AluOpType.mult,
                    accum_out=n_p[:])
                s_parts.append(s_p)
                n_parts.append(n_p)

            out_sb = sb.tile([128, 2], F32, tag="out")
            nc.vector.tensor_add(out_sb[:, 0:1], n_parts[0][:], n_parts[1][:])
            nc.vector.tensor_add(out_sb[:, 1:2], s_parts[0][:], s_parts[1][:])
            nc.sync.dma_start(nd[:], out_sb[:])

    nc.compile()
    return nc


_CACHE = {}

# Filled by kernel() on every call: [("phase1", BassKernelResults), ...].
# exec_time_ns is populated when the KERNEL_PROFILE env var is set.
LAST_RESULTS = []


def _get_programs():
    if "nc1" not in _CACHE:
        _CACHE["nc1"] = _build_phase1()
        _CACHE["nc2"] = _build_phase2()
    return _CACHE["nc1"], _CACHE["nc2"]


def _run(nc, in_maps, name):
    kw = {}
    if os.environ.get("KERNEL_PROFILE"):
        kw = dict(trace=True)
        d = os.environ.get("KERNEL_PROFILE_DIR")
        if d:
            kw["tmpdir"] = os.path.join(d, name)
            os.makedirs(kw["tmpdir"], exist_ok=True)
    res = run_bass_kernel_spmd(nc, in_maps, core_ids=list(range(NCORES)), **kw)
    LAST_RESULTS.append((name, res))
    return res


def _make_idx(tok):
    """tok: [128, L] token ids for one core -> [128, L*8] int16 index tile
    in the wrap-by-16 layout dma_gather expects (replicated to 128 rows)."""
    steps = tok.shape[1]
    a = (tok.astype(np.int16).reshape(8, 16, steps).transpose(1, 2, 0)
         .reshape(16, steps * 8))
    return np.ascontiguousarray(np.tile(a, (8, 1)))


def kernel(sequence, labels, starts, W_ih, W_hh, b_ih, b_hh, proj_W, proj_b):
    bf16 = ml_dtypes.bfloat16
    sequence = np.asarray(sequence)
    labels = np.asarray(labels)
    starts = np.asarray(starts)
    W_ih = np.asarray(W_ih, np.float32)
    W_hh = np.asarray(W_hh, np.float32)
    b_ih = np.asarray(b_ih, np.float32)
    b_hh = np.asarray(b_hh, np.float32)
    proj_W = np.asarray(proj_W, np.float32)
    proj_b = np.asarray(proj_b, np.float32)

    nc1, nc2 = _get_programs()

    # ---- host: subsequence extraction + sharding (pure indexing) ----
    idx = starts[:, :, None].astype(np.int64) + np.arange(L)[None, None, :]
    sub = sequence[np.arange(B)[None, :, None], idx].reshape(N, L)
    lab = np.tile(labels, K)

    bcomb = np.concatenate([
        b_ih[:H] + b_hh[:H], b_ih[H:2 * H] + b_hh[H:2 * H], b_ih[2 * H:]
    ]).astype(np.float32)
    shared = dict(
        table=np.ascontiguousarray((W_ih.T + bcomb[None, :]).astype(bf16)),
        whht=np.ascontiguousarray(W_hh.T.astype(bf16)),
        bhhn=np.ascontiguousarray(b_hh[2 * H:].reshape(1, H).astype(bf16)),
        projwt=np.ascontiguousarray((proj_W.T / L).astype(bf16)),
    )
    in_maps1 = []
    for c in range(NCORES):
        m = dict(shared)
        m["idx"] = _make_idx(sub[c * NLOC:(c + 1) * NLOC, :])
        in_maps1.append(m)

    LAST_RESULTS.clear()
    res1 = _run(nc1, in_maps1, "phase1")
    z = np.concatenate([res1.results[c]["zraw"] for c in range(NCORES)], 0)
    z = z + proj_b[None, :]

    # ---- host glue: normalize + phase-2 operands ----
    norm = np.maximum(np.sqrt((z ** 2).sum(1, keepdims=True)), 1e-12)
    zn = (z / norm).astype(np.float32)
    znb = zn.astype(bf16)
    znt_in = np.ascontiguousarray(znb.T).reshape(2, 128, N)
    pos = (lab[None, :] == lab[:, None]) & ~np.eye(N, dtype=bool)
    posf = pos.astype(bf16)

    in_maps2 = []
    for c in range(NCORES):
        in_maps2.append(dict(
            znt=znt_in,
            zntl=np.ascontiguousarray(znt_in[:, :, c * NLOC:(c + 1) * NLOC]),
            posm=posf[c * NLOC:(c + 1) * NLOC, :],
        ))
    res2 = _run(nc2, in_maps2, "phase2")
    nd = np.concatenate([res2.results[c]["nd"] for c in range(NCORES)], 0)

    num = nd[:, 0].astype(np.float64)
    ssum = nd[:, 1].astype(np.float64)
    zb64 = znb.astype(np.float64)
    es_ii = np.exp((zb64 * zb64).sum(1) / TAU)
    den = ssum - es_ii
    has_pos = pos.any(1)
    li = -np.log(num / (den + 1e-10) + 1e-10)
    loss = np.where(has_pos, li, 0.0).sum() / max(int(has_pos.sum()), 1)
    return np.float32(loss)
